# revision 19
# baseline (speedup 1.0000x reference)
"""Trainium2 Bass kernel for nn_BezierParameterProcessor (raw Bass).

Data-parallel over batch: B=16 -> 2 batches per core on 8 cores.

The KDE gaussian over the [-1,1]^2 tensor-product grid is separable:
    exp(-((gx-px)^2+(gy-py)^2)/(2 bw^2)) = Ex[n,w] * Ey[n,h]
so density/field reduce to per-h-chunk matmuls contracting n:
    dens[h,w]    = sum_n Ey[n,h] *  Ex[n,w]
    field_c[h,w] = sum_n Ey[n,h] * (Ex[n,w] * c0*valid[n]*vecs[n,c])
This needs 2*N*256 exps per batch instead of N*65536, and the whole
[B,HW,N] intermediate never exists. sigmoid(z) = 0.5*tanh(z/2)+0.5 keeps
every ACT call in the single `exp_and_others` table set (exp, tanh,
square all live there -> one ACT_TABLE_LOAD, pre-triggered by a dummy
activation so the ~1.3us load overlaps the input DMA).

The third MLP layer is folded into the field projection on the host
(vecs = Wf@(W3@h2 + b3) + bf = (Wf@W3)@h2 + (bf + Wf@b3)), removing two
serial hops from the device critical path.

fp16 is used for matmul operands (fp32 PSUM accumulation): the gaussian
factors live in [0,1] and the MLP activations are O(1), so fp16 costs
~5e-4 relative error while running the PE single-pass.

Raw Bass (no Tile framework): hand-placed engine programs with explicit
semaphores.  Engine roles:
  SP     input DMAs (3 queues), batch-1 output DMAs (HWDGE)
  ACT    b32y DMA, squares/exps, tanh epilogue
  DVE    relus, (gx-px)^2, Ex*u products
  PE     MLP + vecs + reduction matmuls
  GPSIMD memsets, valid-mask chain, dens sigmoid fixups, and batch-0
         output DMAs on the separate SWDGE path (parallel bandwidth)

PSUM bank map: A(1) = mlp1 out + vecs, B(1) = mlp2 out, R0(3), R1(3).
Per-batch psum R: bank0 = ch0 (dens|f0), bank1 = ch1 (dens|f0),
bank2 = f1 — ACT reads finished banks while the PE fills bank2.
"""

import math
from contextlib import ExitStack

import numpy as np

import concourse.bass as bass
from concourse import mybir
from concourse.bass_utils import run_bass_kernel_spmd

H = W = 256
HID = 128
B = 16
N = 128  # points per batch (C*P = 16*8)
NCORES = 8
BS = B // NCORES  # batches per core = 2

FP32 = mybir.dt.float32
FP16 = mybir.dt.float16

# b32y column layout: gy | npc (negated coords) | bf_eff
C_GY, C_PC, C_BF = 0, 256, 260
NCY = 262
# b16a column layout ([3, .]): w1t | x0
C_W1, C_X0 = 0, 64
NCA = 64 + BS * N
# b16b column layout: w2t | wfoldt
C_W2, C_WF = 0, 128
NCB = 130

LAST_RESULT = None  # BassKernelResults of the most recent run (for profiling)


def _build(neg_inv2bw2, c0, sig_half_scale, sig_half_bias):
    AL = mybir.AluOpType
    ACT = mybir.ActivationFunctionType
    nc = bass.Bass("TRN2", target_bir_lowering=False)

    by_d = nc.declare_dram_parameter("b32y", [128, NCY], FP32, isOutput=False)
    bx_d = nc.declare_dram_parameter("b32x", [128, W], FP32, isOutput=False)
    ba_d = nc.declare_dram_parameter("b16a", [3, NCA], FP16, isOutput=False)
    bb_d = nc.declare_dram_parameter("b16b", [128, NCB], FP16, isOutput=False)
    # scratch layout: [b][partition][1536] = dens(2x256) | f0(2x256) | f1(2x256)
    out_d = nc.declare_dram_parameter("out_o", [BS, 128, 1536], FP32, isOutput=True)

    ctx = ExitStack()
    with ctx:
        sb = lambda nm, shape, dt: ctx.enter_context(nc.sbuf_tensor(nm, shape, dt))
        ps = lambda nm, shape: ctx.enter_context(nc.psum_tensor(nm, shape, FP32))
        sem = lambda name: ctx.enter_context(nc.semaphore(name))

        b32y = sb("b32y_s", [128, NCY], FP32)
        b32x = sb("b32x_s", [128, W], FP32)
        b16a = sb("b16a_s", [3, NCA], FP16)
        b16b = sb("b16b_s", [128, NCB], FP16)
        warm = sb("warm", [1, 1], FP32)
        sigb = sb("sigb", [128, 1], FP32)
        h1 = sb("h1", [65, BS * N], FP16)
        h2 = sb("h2", [HID, BS * N], FP16)
        vb = [sb(f"vb{b}", [N, 2], FP32) for b in range(BS)]
        vc = [sb(f"vcv{b}", [N, 2], FP32) for b in range(BS)]
        ub = [sb(f"ub{b}", [N, 2], FP32) for b in range(BS)]
        txs = sb("txs", [128, BS * W], FP32)
        tys = sb("tys", [128, BS * H], FP32)
        ey = sb("ey", [128, BS * H], FP16)
        exa = [sb(f"exa{b}", [128, 2 * W], FP16) for b in range(BS)]
        exu1 = [sb(f"exu1_{b}", [128, W], FP16) for b in range(BS)]
        obuf = [sb(f"obuf{b}", [128, 1536], FP32) for b in range(BS)]

        psA = ps("psA", [128, BS * N])  # mlp1 out, then vecs at cols 0:2 / 4:6
        psB = ps("psB", [128, BS * N])  # mlp2 out
        psR = [ps(f"psR{b}", [128, 1536]) for b in range(BS)]

        sBY, sBX, sBA, sBB = sem("sBY"), sem("sBX"), sem("sBA"), sem("sBB")
        sPE, sACT, sDVE = sem("sPE"), sem("sACT"), sem("sDVE")
        sGP, sOUT, sOUTG = sem("sGP"), sem("sOUT"), sem("sOUTG")

        gy = b32y[:, C_GY : C_GY + H]
        bfe = b32y[:, C_BF : C_BF + 2]
        npc = lambda b, c: b32y[:, C_PC + 2 * b + c : C_PC + 2 * b + c + 1]
        gx = b32x[:, :]
        w1 = b16a[:3, C_W1 : C_W1 + 64]
        x0 = b16a[:3, C_X0 : C_X0 + BS * N]
        w2 = b16b[:65, C_W2 : C_W2 + HID]
        wfold = b16b[:, C_WF : C_WF + 2]

        with nc.Block() as block:

            @block.gpsimd
            def _(gp):
                gp.memset(warm[:], 0.0).then_inc(sGP)  # GP1: dummy-act operand
                gp.memset(h1[64:65, :], 1.0).then_inc(sGP)  # GP2: layer-2 bias row
                gp.memset(sigb[:], sig_half_bias).then_inc(sGP)  # GP3
                # valid mask, fully off the DVE critical path: GP4..7 / GP8..11
                gp.wait_ge(sBY, 16)
                for b in range(BS):
                    npx, npy = npc(b, 0), npc(b, 1)
                    nc.gpsimd.tensor_scalar(
                        vc[b][:, 0:1], npx, -1.0, npx, AL.mult, AL.max
                    ).then_inc(sGP)
                    nc.gpsimd.tensor_scalar(
                        vc[b][:, 1:2], npy, -1.0, npy, AL.mult, AL.max
                    ).then_inc(sGP)
                    gp.wait_ge(sGP, 5 + 4 * b)
                    nc.gpsimd.tensor_scalar(
                        vc[b][:, 0:1], vc[b][:, 0:1], vc[b][:, 1:2], None, AL.max
                    ).then_inc(sGP)
                    gp.wait_ge(sGP, 6 + 4 * b)
                    nc.gpsimd.tensor_scalar(
                        vc[b][:, 0:1], vc[b][:, 0:1], 1e-8, c0, AL.is_gt, AL.mult
                    ).then_inc(sGP)
                # batch-0 output DMAs (SWDGE path, parallel to HWDGE)
                gp.wait_ge(sDVE, 11)  # dens fixup b0 (on DVE) done
                gp.dma_start(out=out_d[0, :, 0:512], in_=obuf[0][:, 0:512]).then_inc(
                    sOUTG, 16
                )
                gp.wait_ge(sACT, 10)  # f0 tanh b0 done
                gp.dma_start(
                    out=out_d[0, :, 512:1024], in_=obuf[0][:, 512:1024]
                ).then_inc(sOUTG, 16)
                gp.wait_ge(sACT, 11)  # f1 tanh b0 done
                gp.dma_start(
                    out=out_d[0, :, 1024:1536], in_=obuf[0][:, 1024:1536]
                ).then_inc(sOUTG, 16)

            @block.scalar
            def _(act):
                act.dma_start(out=b32y[:], in_=by_d[:]).then_inc(sBY, 16)
                # pre-trigger the exp_and_others ACT_TABLE_LOAD while the
                # input DMAs are in flight
                act.wait_ge(sGP, 1)
                nc.scalar.activation(warm[:], warm[:], ACT.Exp)
                act.wait_ge(sBY, 16)
                # per-batch: sqy(1+4b), sqx(2+4b), ey(3+4b), exa(4+4b) — batch 0's
                # matmul operands (ey half + Ex half) complete ~1.5us earlier
                for b in range(BS):
                    nc.scalar.activation(
                        tys[:, b * H : (b + 1) * H], gy, ACT.Square, bias=npc(b, 1)
                    ).then_inc(sACT)
                    if b == 0:
                        act.wait_ge(sBX, 16)
                    nc.scalar.activation(
                        txs[:, b * W : (b + 1) * W], gx, ACT.Square, bias=npc(b, 0)
                    ).then_inc(sACT)
                    act.wait_ge(sACT, 2 + 4 * b)  # squares landed (same-engine RAW)
                    nc.scalar.activation(
                        ey[:, b * H : (b + 1) * H],
                        tys[:, b * H : (b + 1) * H],
                        ACT.Exp,
                        scale=neg_inv2bw2,
                    ).then_inc(sACT)
                    nc.scalar.activation(
                        exa[b][:, 0:W],
                        txs[:, b * W : (b + 1) * W],
                        ACT.Exp,
                        scale=neg_inv2bw2,
                    ).then_inc(sACT)
                # epilogue: sigmoid(s*x - thr) = 0.5*tanh(s/2*x - thr/2) + 0.5
                act.wait_ge(sGP, 3)
                for b in range(BS):  # A8,A9,A10 / A11,A12,A13
                    pr = psR[b][:, 0:1024].rearrange("p (c x) -> p c x", c=2)
                    act.wait_ge(sPE, 5 + 2 * b)  # dens|f0 matmuls of batch b
                    nc.scalar.activation(
                        obuf[b][:, 0:512].rearrange("p (c w) -> p c w", c=2),
                        pr[:, :, 0:W],
                        ACT.Tanh,
                        scale=sig_half_scale,
                        bias=sigb[:],
                    ).then_inc(sACT)
                    nc.scalar.activation(
                        obuf[b][:, 512:1024].rearrange("p (c w) -> p c w", c=2),
                        pr[:, :, W : 2 * W],
                        ACT.Tanh,
                    ).then_inc(sACT)
                    act.wait_ge(sPE, 6 + 2 * b)  # f1 matmuls of batch b
                    nc.scalar.activation(
                        obuf[b][:, 1024:1536], psR[b][:, 1024:1536], ACT.Tanh
                    ).then_inc(sACT)
                if True:  # ship f1(b1) from here: no cross-engine hop at the tail
                    act.wait_ge(sACT, 14)
                    act.dma_start(
                        out=out_d[1, :, 1024:1280], in_=obuf[1][:, 1024:1280]
                    ).then_inc(sOUT, 16)
                    act.dma_start(
                        out=out_d[1, :, 1280:1536], in_=obuf[1][:, 1280:1536]
                    ).then_inc(sOUT, 16)

            @block.vector
            def _(dve):
                dve.wait_ge(sPE, 1)
                nc.vector.tensor_scalar(
                    h1[:64, :], psA[:64, :], 0.0, None, AL.max
                ).then_inc(sDVE)  # D1: relu1
                dve.wait_ge(sPE, 2)
                nc.vector.tensor_scalar(h2[:], psB[:], 0.0, None, AL.max).then_inc(
                    sDVE
                )  # D2: relu2
                dve.wait_ge(sBY, 16)
                for b in range(BS):  # D3,D4 / D5,D6
                    dve.wait_ge(sPE, 3 + b)
                    nc.vector.tensor_tensor(
                        vb[b][:], psA[:N, 4 * b : 4 * b + 2], bfe, AL.add
                    ).then_inc(sDVE)
                    dve.wait_ge(sDVE, 3 + 2 * b)
                    dve.wait_ge(sGP, 7 + 4 * b)  # vc[b] ready (on gpsimd)
                    nc.vector.tensor_scalar(
                        ub[b][:], vb[b][:], vc[b][:, 0:1], None, AL.mult
                    ).then_inc(sDVE)
                for b in range(BS):  # D7,D8 / D9,D10
                    dve.wait_ge(sDVE, 4 + 2 * b)  # ub[b] landed
                    dve.wait_ge(sACT, 4 + 4 * b)
                    nc.vector.tensor_scalar(
                        exa[b][:, W : 2 * W],
                        exa[b][:, 0:W],
                        ub[b][:, 0:1],
                        None,
                        AL.mult,
                    ).then_inc(sDVE)
                    nc.vector.tensor_scalar(
                        exu1[b][:], exa[b][:, 0:W], ub[b][:, 1:2], None, AL.mult
                    ).then_inc(sDVE)
                # dens sigmoid fixups
                dve.wait_ge(sACT, 9)  # dens tanh b0 done
                nc.vector.tensor_scalar(
                    obuf[0][:, 0:512], obuf[0][:, 0:512], 0.5, 0.5, AL.mult, AL.add
                ).then_inc(sDVE)  # D11: fix0
                dve.wait_ge(sACT, 12)  # dens tanh b1 done
                nc.vector.tensor_scalar(
                    obuf[1][:, 0:512], obuf[1][:, 0:512], 0.5, 0.5, AL.mult, AL.add
                ).then_inc(sDVE)  # D12: fix1

            @block.tensor
            def _(pe):
                pe.wait_ge(sBA, 16)
                nc.tensor.matmul(psA[:64, :], w1, x0).then_inc(sPE)  # P1
                pe.wait_ge(sDVE, 1)
                pe.wait_ge(sGP, 2)
                pe.wait_ge(sBB, 16)
                nc.tensor.matmul(psB[:], w2, h1[:]).then_inc(sPE)  # P2
                for b in range(BS):  # P3, P4: vecs (pre-bias) via folded Wf@W3
                    # psA bank0 is read by DVE (relu1, vb[b-1]) — wait for the
                    # read to finish before writing the same bank (P10 hazard)
                    pe.wait_ge(sDVE, 2 + b)
                    nc.tensor.matmul(
                        psA[:N, 4 * b : 4 * b + 2], h2[:, b * N : (b + 1) * N], wfold
                    ).then_inc(sPE)
                for b in range(BS):
                    pe.wait_ge(sACT, 4 + 4 * b)  # exa[b] Ex half (implies ey[b])
                    pe.wait_ge(sDVE, 8 + 2 * b)  # exa[b] U-half + exu1[b]
                    for ch in range(2):  # dens|f0 fused, N=512
                        lhs = ey[:, b * H + ch * 128 : b * H + (ch + 1) * 128]
                        mm = nc.tensor.matmul(
                            psR[b][:, ch * 512 : (ch + 1) * 512], lhs, exa[b][:]
                        )
                        if ch == 1:
                            mm.then_inc(sPE)  # -> 5 + 2b
                    for ch in range(2):  # f1
                        lhs = ey[:, b * H + ch * 128 : b * H + (ch + 1) * 128]
                        mm = nc.tensor.matmul(
                            psR[b][:, 1024 + ch * W : 1024 + (ch + 1) * W],
                            lhs,
                            exu1[b][:],
                        )
                        if ch == 1:
                            mm.then_inc(sPE)  # -> 6 + 2b

            @block.sync
            def _(sp):
                sp.dma_start(out=b16a[:], in_=ba_d[:]).then_inc(sBA, 16)
                sp.dma_start(out=b32x[:], in_=bx_d[:]).then_inc(sBX, 16)
                sp.dma_start(out=b16b[:], in_=bb_d[:]).then_inc(sBB, 16)
                # batch-1 outputs (batch 0 ships from gpsimd/SWDGE)
                sp.wait_ge(sDVE, 12)  # dens fixup b1 done
                sp.dma_start(out=out_d[1, :, 0:512], in_=obuf[1][:, 0:512]).then_inc(
                    sOUT, 16
                )
                sp.wait_ge(sACT, 13)  # f0 tanh b1 done
                sp.dma_start(
                    out=out_d[1, :, 512:1024], in_=obuf[1][:, 512:1024]
                ).then_inc(sOUT, 16)
                sp.wait_ge(sOUT, 16 * 4)
                sp.wait_ge(sOUTG, 16 * 3)

    return nc


def kernel(
    bezier_points,
    W1,
    b1,
    W2,
    b2,
    W3,
    b3,
    Wf,
    bf,
    kde_bandwidth,
    density_threshold,
    trace=False,
):
    global LAST_RESULT
    f32, f16 = np.float32, np.float16
    pts = np.asarray(bezier_points, f32).reshape(B, N, 2)
    W1, b1 = np.asarray(W1, f32), np.asarray(b1, f32)
    W2, b2 = np.asarray(W2, f32), np.asarray(b2, f32)
    W3, b3 = np.asarray(W3, f32), np.asarray(b3, f32)
    Wf, bf = np.asarray(Wf, f32), np.asarray(bf, f32)

    bw = max(float(np.float32(kde_bandwidth)), 1e-5)
    thr = float(np.float32(density_threshold))
    neg_inv2bw2 = -1.0 / (2.0 * bw * bw)
    c0 = math.exp(-1e-8 / (2.0 * bw * bw))
    s = 1.0 / (N * bw * math.sqrt(2.0 * math.pi))
    nc = _build(neg_inv2bw2, c0, 0.5 * s, -0.5 * thr)

    # host-side input marshalling: grid constants, weight transposes, and
    # folding layer 3 into the field projection
    wfold = (Wf @ W3).astype(f32)  # [2, HID]
    bf_eff = (bf + Wf @ b3).astype(f32)  # [2]

    bloby = np.zeros((128, NCY), f32)
    bloby[:, C_GY : C_GY + H] = np.linspace(-1.0, 1.0, H, dtype=f32)
    bloby[:, C_BF : C_BF + 2] = bf_eff
    blobx = np.ascontiguousarray(
        np.broadcast_to(np.linspace(-1.0, 1.0, W, dtype=f32), (128, W))
    )
    bloba = np.zeros((3, NCA), f16)
    bloba[:, C_W1 : C_W1 + 64] = np.vstack([W1.T, b1[None, :]])
    blobb = np.zeros((128, NCB), f16)
    blobb[:65, C_W2 : C_W2 + HID] = np.vstack([W2.T, b2[None, :]])
    blobb[:, C_WF : C_WF + 2] = wfold.T

    in_maps = []
    for i in range(NCORES):
        sh = pts[i * BS : (i + 1) * BS]  # [BS, N, 2]
        cy = bloby.copy()
        for b in range(BS):
            cy[:, C_PC + 2 * b] = -sh[b, :, 0]
            cy[:, C_PC + 2 * b + 1] = -sh[b, :, 1]
        ca = bloba.copy()
        ca[0, C_X0 : C_X0 + BS * N] = sh[..., 0].reshape(-1)
        ca[1, C_X0 : C_X0 + BS * N] = sh[..., 1].reshape(-1)
        ca[2, C_X0 : C_X0 + BS * N] = 1.0
        in_maps.append({"b32y": cy, "b32x": blobx, "b16a": ca, "b16b": blobb})

    res = run_bass_kernel_spmd(nc, in_maps, list(range(NCORES)), trace=trace)
    LAST_RESULT = res

    density = np.empty((B, 1, H, W), f32)
    field = np.empty((B, 2, H, W), f32)
    for i in range(NCORES):
        scr = res.results[i]["out_o"]  # [BS, 128, 1536]
        maps = scr.reshape(BS, 128, 3, 2, W).transpose(2, 0, 3, 1, 4)
        # maps[m, b, ch, p, w] with h = ch*128 + p
        density[i * BS : (i + 1) * BS, 0] = maps[0].reshape(BS, H, W)
        field[i * BS : (i + 1) * BS, 0] = maps[1].reshape(BS, H, W)
        field[i * BS : (i + 1) * BS, 1] = maps[2].reshape(BS, H, W)
    return density, field


# revision 20
# speedup vs baseline: 1.0163x; 1.0163x over previous
"""Trainium2 Bass kernel for nn_BezierParameterProcessor (raw Bass).

Data-parallel over batch: B=16 -> 2 batches per core on 8 cores.

The KDE gaussian over the [-1,1]^2 tensor-product grid is separable:
    exp(-((gx-px)^2+(gy-py)^2)/(2 bw^2)) = Ex[n,w] * Ey[n,h]
so density/field reduce to per-h-chunk matmuls contracting n:
    dens[h,w]    = sum_n Ey[n,h] *  Ex[n,w]
    field_c[h,w] = sum_n Ey[n,h] * (Ex[n,w] * c0*valid[n]*vecs[n,c])
This needs 2*N*256 exps per batch instead of N*65536, and the whole
[B,HW,N] intermediate never exists. sigmoid(z) = 0.5*tanh(z/2)+0.5 keeps
every ACT call in the single `exp_and_others` table set (exp, tanh,
square all live there -> one ACT_TABLE_LOAD, pre-triggered by a dummy
activation so the ~1.3us load overlaps the input DMA).

The third MLP layer is folded into the field projection on the host
(vecs = Wf@(W3@h2 + b3) + bf = (Wf@W3)@h2 + (bf + Wf@b3)), removing two
serial hops from the device critical path.

fp16 is used for matmul operands (fp32 PSUM accumulation): the gaussian
factors live in [0,1] and the MLP activations are O(1), so fp16 costs
~5e-4 relative error while running the PE single-pass.

Raw Bass (no Tile framework): hand-placed engine programs with explicit
semaphores.  Engine roles:
  SP     input DMAs (3 queues), batch-1 output DMAs (HWDGE)
  ACT    b32y DMA, squares/exps, tanh epilogue
  DVE    relus, (gx-px)^2, Ex*u products
  PE     MLP + vecs + reduction matmuls
  GPSIMD memsets, valid-mask chain, dens sigmoid fixups, and batch-0
         output DMAs on the separate SWDGE path (parallel bandwidth)

PSUM bank map: A(1) = mlp1 out + vecs, B(1) = mlp2 out, R0(3), R1(3).
Per-batch psum R: bank0 = ch0 (dens|f0), bank1 = ch1 (dens|f0),
bank2 = f1 — ACT reads finished banks while the PE fills bank2.
"""

import math
from contextlib import ExitStack

import numpy as np

import concourse.bass as bass
from concourse import mybir
from concourse.bass_utils import run_bass_kernel_spmd

H = W = 256
HID = 128
B = 16
N = 128  # points per batch (C*P = 16*8)
NCORES = 8
BS = B // NCORES  # batches per core = 2

FP32 = mybir.dt.float32
FP16 = mybir.dt.float16

# b32y column layout: gy | npc (negated coords) | bf_eff
C_GY, C_PC, C_BF = 0, 256, 260
NCY = 262
# b16a column layout ([3, .]): w1t | x0
C_W1, C_X0 = 0, 64
NCA = 64 + BS * N
# b16b column layout: w2t | wfoldt
C_W2, C_WF = 0, 128
NCB = 130

LAST_RESULT = None  # BassKernelResults of the most recent run (for profiling)


def _build(neg_inv2bw2, c0, sig_half_scale, sig_half_bias):
    AL = mybir.AluOpType
    ACT = mybir.ActivationFunctionType
    nc = bass.Bass("TRN2", target_bir_lowering=False)

    by_d = nc.declare_dram_parameter("b32y", [128, NCY], FP32, isOutput=False)
    bx_d = nc.declare_dram_parameter("b32x", [128, W], FP32, isOutput=False)
    ba_d = nc.declare_dram_parameter("b16a", [3, NCA], FP16, isOutput=False)
    bb_d = nc.declare_dram_parameter("b16b", [128, NCB], FP16, isOutput=False)
    # scratch layout: [b][partition][1536] = dens(2x256) | f0(2x256) | f1(2x256)
    out_d = nc.declare_dram_parameter("out_o", [BS, 128, 1536], FP32, isOutput=True)

    ctx = ExitStack()
    with ctx:
        sb = lambda nm, shape, dt: ctx.enter_context(nc.sbuf_tensor(nm, shape, dt))
        ps = lambda nm, shape: ctx.enter_context(nc.psum_tensor(nm, shape, FP32))
        sem = lambda name: ctx.enter_context(nc.semaphore(name))

        b32y = sb("b32y_s", [128, NCY], FP32)
        b32x = sb("b32x_s", [128, W], FP32)
        b16a = sb("b16a_s", [3, NCA], FP16)
        b16b = sb("b16b_s", [128, NCB], FP16)
        warm = sb("warm", [1, 1], FP32)
        sigb = sb("sigb", [128, 1], FP32)
        h1 = sb("h1", [65, BS * N], FP16)
        h2 = sb("h2", [HID, BS * N], FP16)
        vb = [sb(f"vb{b}", [N, 2], FP32) for b in range(BS)]
        vc = [sb(f"vcv{b}", [N, 2], FP32) for b in range(BS)]
        ub = [sb(f"ub{b}", [N, 2], FP32) for b in range(BS)]
        txs = sb("txs", [128, BS * W], FP32)
        tys = sb("tys", [128, BS * H], FP32)
        ey = sb("ey", [128, BS * H], FP16)
        exa = [sb(f"exa{b}", [128, 2 * W], FP16) for b in range(BS)]
        exu1 = [sb(f"exu1_{b}", [128, W], FP16) for b in range(BS)]
        obuf = [sb(f"obuf{b}", [128, 1536], FP32) for b in range(BS)]

        psA = ps("psA", [128, BS * N])  # mlp1 out, then vecs at cols 0:2 / 4:6
        psB = ps("psB", [128, BS * N])  # mlp2 out
        psR = [ps(f"psR{b}", [128, 1536]) for b in range(BS)]

        sBY, sBX, sBA, sBB = sem("sBY"), sem("sBX"), sem("sBA"), sem("sBB")
        sPE, sACT, sDVE = sem("sPE"), sem("sACT"), sem("sDVE")
        sGP, sOUT, sOUTG = sem("sGP"), sem("sOUT"), sem("sOUTG")

        gy = b32y[:, C_GY : C_GY + H]
        bfe = b32y[:, C_BF : C_BF + 2]
        npc = lambda b, c: b32y[:, C_PC + 2 * b + c : C_PC + 2 * b + c + 1]
        gx = b32x[:, :]
        w1 = b16a[:3, C_W1 : C_W1 + 64]
        x0 = b16a[:3, C_X0 : C_X0 + BS * N]
        w2 = b16b[:65, C_W2 : C_W2 + HID]
        wfold = b16b[:, C_WF : C_WF + 2]

        with nc.Block() as block:

            @block.gpsimd
            def _(gp):
                gp.memset(warm[:], 0.0).then_inc(sGP)  # GP1: dummy-act operand
                gp.memset(h1[64:65, :], 1.0).then_inc(sGP)  # GP2: layer-2 bias row
                gp.memset(sigb[:], sig_half_bias).then_inc(sGP)  # GP3
                # valid mask, fully off the DVE critical path: GP4..7 / GP8..11
                gp.wait_ge(sBY, 16)
                for b in range(BS):
                    npx, npy = npc(b, 0), npc(b, 1)
                    nc.gpsimd.tensor_scalar(
                        vc[b][:, 0:1], npx, -1.0, npx, AL.mult, AL.max
                    ).then_inc(sGP)
                    nc.gpsimd.tensor_scalar(
                        vc[b][:, 1:2], npy, -1.0, npy, AL.mult, AL.max
                    ).then_inc(sGP)
                    gp.wait_ge(sGP, 5 + 4 * b)
                    nc.gpsimd.tensor_scalar(
                        vc[b][:, 0:1], vc[b][:, 0:1], vc[b][:, 1:2], None, AL.max
                    ).then_inc(sGP)
                    gp.wait_ge(sGP, 6 + 4 * b)
                    nc.gpsimd.tensor_scalar(
                        vc[b][:, 0:1], vc[b][:, 0:1], 1e-8, c0, AL.is_gt, AL.mult
                    ).then_inc(sGP)
                # batch-0 field DMAs (SWDGE path, parallel to HWDGE)
                gp.wait_ge(sACT, 10)  # f0 tanh b0 done
                gp.dma_start(
                    out=out_d[0, :, 512:1024], in_=obuf[0][:, 512:1024]
                ).then_inc(sOUTG, 16)
                gp.wait_ge(sACT, 11)  # f1 tanh b0 done
                gp.dma_start(
                    out=out_d[0, :, 1024:1536], in_=obuf[0][:, 1024:1536]
                ).then_inc(sOUTG, 16)

            @block.scalar
            def _(act):
                act.dma_start(out=b32y[:], in_=by_d[:]).then_inc(sBY, 16)
                # pre-trigger the exp_and_others ACT_TABLE_LOAD while the
                # input DMAs are in flight
                act.wait_ge(sGP, 1)
                nc.scalar.activation(warm[:], warm[:], ACT.Exp)
                act.wait_ge(sBY, 16)
                # per-batch: sqy(1+4b), sqx(2+4b), ey(3+4b), exa(4+4b) — batch 0's
                # matmul operands (ey half + Ex half) complete ~1.5us earlier
                for b in range(BS):
                    nc.scalar.activation(
                        tys[:, b * H : (b + 1) * H], gy, ACT.Square, bias=npc(b, 1)
                    ).then_inc(sACT)
                    if b == 0:
                        act.wait_ge(sBX, 16)
                    nc.scalar.activation(
                        txs[:, b * W : (b + 1) * W], gx, ACT.Square, bias=npc(b, 0)
                    ).then_inc(sACT)
                    act.wait_ge(sACT, 2 + 4 * b)  # squares landed (same-engine RAW)
                    nc.scalar.activation(
                        ey[:, b * H : (b + 1) * H],
                        tys[:, b * H : (b + 1) * H],
                        ACT.Exp,
                        scale=neg_inv2bw2,
                    ).then_inc(sACT)
                    nc.scalar.activation(
                        exa[b][:, 0:W],
                        txs[:, b * W : (b + 1) * W],
                        ACT.Exp,
                        scale=neg_inv2bw2,
                    ).then_inc(sACT)
                # epilogue: sigmoid(s*x - thr) = 0.5*tanh(s/2*x - thr/2) + 0.5
                act.wait_ge(sGP, 3)
                for b in range(BS):  # A8,A9,A10 / A11,A12,A13
                    pr = psR[b][:, 0:1024].rearrange("p (c x) -> p c x", c=2)
                    act.wait_ge(sPE, 5 + 2 * b)  # dens|f0 matmuls of batch b
                    nc.scalar.activation(
                        obuf[b][:, 0:512].rearrange("p (c w) -> p c w", c=2),
                        pr[:, :, 0:W],
                        ACT.Tanh,
                        scale=sig_half_scale,
                        bias=sigb[:],
                    ).then_inc(sACT)
                    nc.scalar.activation(
                        obuf[b][:, 512:1024].rearrange("p (c w) -> p c w", c=2),
                        pr[:, :, W : 2 * W],
                        ACT.Tanh,
                    ).then_inc(sACT)
                    act.wait_ge(sPE, 6 + 2 * b)  # f1 matmuls of batch b
                    nc.scalar.activation(
                        obuf[b][:, 1024:1536], psR[b][:, 1024:1536], ACT.Tanh
                    ).then_inc(sACT)
                if True:  # ship f1(b1) from here: no cross-engine hop at the tail
                    act.wait_ge(sACT, 14)
                    act.dma_start(
                        out=out_d[1, :, 1024:1280], in_=obuf[1][:, 1024:1280]
                    ).then_inc(sOUT, 16)
                    act.dma_start(
                        out=out_d[1, :, 1280:1536], in_=obuf[1][:, 1280:1536]
                    ).then_inc(sOUT, 16)

            @block.vector
            def _(dve):
                dve.wait_ge(sPE, 1)
                nc.vector.tensor_scalar(
                    h1[:64, :], psA[:64, :], 0.0, None, AL.max
                ).then_inc(sDVE)  # D1: relu1
                dve.wait_ge(sPE, 2)
                nc.vector.tensor_scalar(h2[:], psB[:], 0.0, None, AL.max).then_inc(
                    sDVE
                )  # D2: relu2
                dve.wait_ge(sBY, 16)
                for b in range(BS):  # D3,D4 / D5,D6
                    dve.wait_ge(sPE, 3 + b)
                    nc.vector.tensor_tensor(
                        vb[b][:], psA[:N, 4 * b : 4 * b + 2], bfe, AL.add
                    ).then_inc(sDVE)
                    dve.wait_ge(sDVE, 3 + 2 * b)
                    dve.wait_ge(sGP, 7 + 4 * b)  # vc[b] ready (on gpsimd)
                    nc.vector.tensor_scalar(
                        ub[b][:], vb[b][:], vc[b][:, 0:1], None, AL.mult
                    ).then_inc(sDVE)
                for b in range(BS):  # D7,D8 / D9,D10
                    dve.wait_ge(sDVE, 4 + 2 * b)  # ub[b] landed
                    dve.wait_ge(sACT, 4 + 4 * b)
                    nc.vector.tensor_scalar(
                        exa[b][:, W : 2 * W],
                        exa[b][:, 0:W],
                        ub[b][:, 0:1],
                        None,
                        AL.mult,
                    ).then_inc(sDVE)
                    nc.vector.tensor_scalar(
                        exu1[b][:], exa[b][:, 0:W], ub[b][:, 1:2], None, AL.mult
                    ).then_inc(sDVE)
                # dens sigmoid fixups
                dve.wait_ge(sACT, 9)  # dens tanh b0 done
                nc.vector.tensor_scalar(
                    obuf[0][:, 0:512], obuf[0][:, 0:512], 0.5, 0.5, AL.mult, AL.add
                ).then_inc(sDVE)  # D11: fix0
                dve.wait_ge(sACT, 12)  # dens tanh b1 done
                nc.vector.tensor_scalar(
                    obuf[1][:, 0:512], obuf[1][:, 0:512], 0.5, 0.5, AL.mult, AL.add
                ).then_inc(sDVE)  # D12: fix1

            @block.tensor
            def _(pe):
                pe.wait_ge(sBA, 16)
                nc.tensor.matmul(psA[:64, :], w1, x0).then_inc(sPE)  # P1
                pe.wait_ge(sDVE, 1)
                pe.wait_ge(sGP, 2)
                pe.wait_ge(sBB, 16)
                nc.tensor.matmul(psB[:], w2, h1[:]).then_inc(sPE)  # P2
                for b in range(BS):  # P3, P4: vecs (pre-bias) via folded Wf@W3
                    # psA bank0 is read by DVE (relu1, vb[b-1]) — wait for the
                    # read to finish before writing the same bank (P10 hazard)
                    pe.wait_ge(sDVE, 2 + b)
                    nc.tensor.matmul(
                        psA[:N, 4 * b : 4 * b + 2], h2[:, b * N : (b + 1) * N], wfold
                    ).then_inc(sPE)
                for b in range(BS):
                    pe.wait_ge(sACT, 4 + 4 * b)  # exa[b] Ex half (implies ey[b])
                    pe.wait_ge(sDVE, 8 + 2 * b)  # exa[b] U-half + exu1[b]
                    for ch in range(2):  # dens|f0 fused, N=512
                        lhs = ey[:, b * H + ch * 128 : b * H + (ch + 1) * 128]
                        mm = nc.tensor.matmul(
                            psR[b][:, ch * 512 : (ch + 1) * 512], lhs, exa[b][:]
                        )
                        if ch == 1:
                            mm.then_inc(sPE)  # -> 5 + 2b
                    for ch in range(2):  # f1
                        lhs = ey[:, b * H + ch * 128 : b * H + (ch + 1) * 128]
                        mm = nc.tensor.matmul(
                            psR[b][:, 1024 + ch * W : 1024 + (ch + 1) * W],
                            lhs,
                            exu1[b][:],
                        )
                        if ch == 1:
                            mm.then_inc(sPE)  # -> 6 + 2b

            @block.sync
            def _(sp):
                sp.dma_start(out=b16a[:], in_=ba_d[:]).then_inc(sBA, 16)
                sp.dma_start(out=b32x[:], in_=bx_d[:]).then_inc(sBX, 16)
                sp.dma_start(out=b16b[:], in_=bb_d[:]).then_inc(sBB, 16)
                # dens b0 (SP is idle; avoids the gpsimd hop), then batch 1
                sp.wait_ge(sDVE, 11)  # dens fixup b0 done
                sp.dma_start(out=out_d[0, :, 0:512], in_=obuf[0][:, 0:512]).then_inc(
                    sOUT, 16
                )
                sp.wait_ge(sDVE, 12)  # dens fixup b1 done
                sp.dma_start(out=out_d[1, :, 0:512], in_=obuf[1][:, 0:512]).then_inc(
                    sOUT, 16
                )
                sp.wait_ge(sACT, 13)  # f0 tanh b1 done
                sp.dma_start(
                    out=out_d[1, :, 512:1024], in_=obuf[1][:, 512:1024]
                ).then_inc(sOUT, 16)
                sp.wait_ge(sOUT, 16 * 5)
                sp.wait_ge(sOUTG, 16 * 2)

    return nc


def kernel(
    bezier_points,
    W1,
    b1,
    W2,
    b2,
    W3,
    b3,
    Wf,
    bf,
    kde_bandwidth,
    density_threshold,
    trace=False,
):
    global LAST_RESULT
    f32, f16 = np.float32, np.float16
    pts = np.asarray(bezier_points, f32).reshape(B, N, 2)
    W1, b1 = np.asarray(W1, f32), np.asarray(b1, f32)
    W2, b2 = np.asarray(W2, f32), np.asarray(b2, f32)
    W3, b3 = np.asarray(W3, f32), np.asarray(b3, f32)
    Wf, bf = np.asarray(Wf, f32), np.asarray(bf, f32)

    bw = max(float(np.float32(kde_bandwidth)), 1e-5)
    thr = float(np.float32(density_threshold))
    neg_inv2bw2 = -1.0 / (2.0 * bw * bw)
    c0 = math.exp(-1e-8 / (2.0 * bw * bw))
    s = 1.0 / (N * bw * math.sqrt(2.0 * math.pi))
    nc = _build(neg_inv2bw2, c0, 0.5 * s, -0.5 * thr)

    # host-side input marshalling: grid constants, weight transposes, and
    # folding layer 3 into the field projection
    wfold = (Wf @ W3).astype(f32)  # [2, HID]
    bf_eff = (bf + Wf @ b3).astype(f32)  # [2]

    bloby = np.zeros((128, NCY), f32)
    bloby[:, C_GY : C_GY + H] = np.linspace(-1.0, 1.0, H, dtype=f32)
    bloby[:, C_BF : C_BF + 2] = bf_eff
    blobx = np.ascontiguousarray(
        np.broadcast_to(np.linspace(-1.0, 1.0, W, dtype=f32), (128, W))
    )
    bloba = np.zeros((3, NCA), f16)
    bloba[:, C_W1 : C_W1 + 64] = np.vstack([W1.T, b1[None, :]])
    blobb = np.zeros((128, NCB), f16)
    blobb[:65, C_W2 : C_W2 + HID] = np.vstack([W2.T, b2[None, :]])
    blobb[:, C_WF : C_WF + 2] = wfold.T

    in_maps = []
    for i in range(NCORES):
        sh = pts[i * BS : (i + 1) * BS]  # [BS, N, 2]
        cy = bloby.copy()
        for b in range(BS):
            cy[:, C_PC + 2 * b] = -sh[b, :, 0]
            cy[:, C_PC + 2 * b + 1] = -sh[b, :, 1]
        ca = bloba.copy()
        ca[0, C_X0 : C_X0 + BS * N] = sh[..., 0].reshape(-1)
        ca[1, C_X0 : C_X0 + BS * N] = sh[..., 1].reshape(-1)
        ca[2, C_X0 : C_X0 + BS * N] = 1.0
        in_maps.append({"b32y": cy, "b32x": blobx, "b16a": ca, "b16b": blobb})

    res = run_bass_kernel_spmd(nc, in_maps, list(range(NCORES)), trace=trace)
    LAST_RESULT = res

    density = np.empty((B, 1, H, W), f32)
    field = np.empty((B, 2, H, W), f32)
    for i in range(NCORES):
        scr = res.results[i]["out_o"]  # [BS, 128, 1536]
        maps = scr.reshape(BS, 128, 3, 2, W).transpose(2, 0, 3, 1, 4)
        # maps[m, b, ch, p, w] with h = ch*128 + p
        density[i * BS : (i + 1) * BS, 0] = maps[0].reshape(BS, H, W)
        field[i * BS : (i + 1) * BS, 0] = maps[1].reshape(BS, H, W)
        field[i * BS : (i + 1) * BS, 1] = maps[2].reshape(BS, H, W)
    return density, field


# revision 21
# speedup vs baseline: 1.0241x; 1.0077x over previous
"""Trainium2 Bass kernel for nn_BezierParameterProcessor (raw Bass).

Data-parallel over batch: B=16 -> 2 batches per core on 8 cores.

The KDE gaussian over the [-1,1]^2 tensor-product grid is separable:
    exp(-((gx-px)^2+(gy-py)^2)/(2 bw^2)) = Ex[n,w] * Ey[n,h]
so density/field reduce to per-h-chunk matmuls contracting n:
    dens[h,w]    = sum_n Ey[n,h] *  Ex[n,w]
    field_c[h,w] = sum_n Ey[n,h] * (Ex[n,w] * c0*valid[n]*vecs[n,c])
This needs 2*N*256 exps per batch instead of N*65536, and the whole
[B,HW,N] intermediate never exists. sigmoid(z) = 0.5*tanh(z/2)+0.5 keeps
every ACT call in the single `exp_and_others` table set (exp, tanh,
square all live there -> one ACT_TABLE_LOAD, pre-triggered by a dummy
activation so the ~1.3us load overlaps the input DMA).

The third MLP layer is folded into the field projection on the host
(vecs = Wf@(W3@h2 + b3) + bf = (Wf@W3)@h2 + (bf + Wf@b3)), removing two
serial hops from the device critical path.

fp16 is used for matmul operands (fp32 PSUM accumulation): the gaussian
factors live in [0,1] and the MLP activations are O(1), so fp16 costs
~5e-4 relative error while running the PE single-pass.

Raw Bass (no Tile framework): hand-placed engine programs with explicit
semaphores.  Engine roles:
  SP     input DMAs (3 queues), batch-1 output DMAs (HWDGE)
  ACT    b32y DMA, squares/exps, tanh epilogue
  DVE    relus, (gx-px)^2, Ex*u products
  PE     MLP + vecs + reduction matmuls
  GPSIMD memsets, valid-mask chain, dens sigmoid fixups, and batch-0
         output DMAs on the separate SWDGE path (parallel bandwidth)

PSUM bank map: A(1) = mlp1 out + vecs, B(1) = mlp2 out, R0(3), R1(3).
Per-batch psum R: bank0 = ch0 (dens|f0), bank1 = ch1 (dens|f0),
bank2 = f1 — ACT reads finished banks while the PE fills bank2.
"""

import math
from contextlib import ExitStack

import numpy as np

import concourse.bass as bass
from concourse import mybir
from concourse.bass_utils import run_bass_kernel_spmd

H = W = 256
HID = 128
B = 16
N = 128  # points per batch (C*P = 16*8)
NCORES = 8
BS = B // NCORES  # batches per core = 2

FP32 = mybir.dt.float32
FP16 = mybir.dt.float16

# b32y column layout: gy | npc (negated coords) | bf_eff
C_GY, C_PC, C_BF = 0, 256, 260
NCY = 262
# b16a column layout ([3, .]): w1t | x0
C_W1, C_X0 = 0, 64
NCA = 64 + BS * N
# b16b column layout: w2t | wfoldt
C_W2, C_WF = 0, 128
NCB = 130

LAST_RESULT = None  # BassKernelResults of the most recent run (for profiling)


def _build(neg_inv2bw2, c0, sig_half_scale, sig_half_bias):
    AL = mybir.AluOpType
    ACT = mybir.ActivationFunctionType
    nc = bass.Bass("TRN2", target_bir_lowering=False)

    by_d = nc.declare_dram_parameter("b32y", [128, NCY], FP32, isOutput=False)
    bx_d = nc.declare_dram_parameter("b32x", [128, W], FP32, isOutput=False)
    ba_d = nc.declare_dram_parameter("b16a", [3, NCA], FP16, isOutput=False)
    bb_d = nc.declare_dram_parameter("b16b", [128, NCB], FP16, isOutput=False)
    # scratch layout: [b][partition][1536] = dens(2x256) | f0(2x256) | f1(2x256)
    out_d = nc.declare_dram_parameter("out_o", [BS, 128, 1536], FP32, isOutput=True)

    ctx = ExitStack()
    with ctx:
        sb = lambda nm, shape, dt: ctx.enter_context(nc.sbuf_tensor(nm, shape, dt))
        ps = lambda nm, shape: ctx.enter_context(nc.psum_tensor(nm, shape, FP32))
        sem = lambda name: ctx.enter_context(nc.semaphore(name))

        b32y = sb("b32y_s", [128, NCY], FP32)
        b32x = sb("b32x_s", [128, W], FP32)
        b16a = sb("b16a_s", [3, NCA], FP16)
        b16b = sb("b16b_s", [128, NCB], FP16)
        warm = sb("warm", [1, 1], FP32)
        sigb = sb("sigb", [128, 1], FP32)
        h1 = sb("h1", [65, BS * N], FP16)
        h2 = sb("h2", [HID, BS * N], FP16)
        vb = [sb(f"vb{b}", [N, 2], FP32) for b in range(BS)]
        vc = [sb(f"vcv{b}", [N, 2], FP32) for b in range(BS)]
        ub = [sb(f"ub{b}", [N, 2], FP32) for b in range(BS)]
        txs = sb("txs", [128, BS * W], FP32)
        tys = sb("tys", [128, BS * H], FP32)
        ey = sb("ey", [128, BS * H], FP16)
        exa = [sb(f"exa{b}", [128, 2 * W], FP16) for b in range(BS)]
        exu1 = [sb(f"exu1_{b}", [128, W], FP16) for b in range(BS)]
        obuf = [sb(f"obuf{b}", [128, 1536], FP32) for b in range(BS)]

        psA = ps("psA", [128, BS * N])  # mlp1 out, then vecs at cols 0:2 / 4:6
        psB = ps("psB", [128, BS * N])  # mlp2 out
        psR = [ps(f"psR{b}", [128, 1536]) for b in range(BS)]

        sBY, sBX, sBA, sBB = sem("sBY"), sem("sBX"), sem("sBA"), sem("sBB")
        sPE, sACT, sDVE = sem("sPE"), sem("sACT"), sem("sDVE")
        sGP, sOUT, sOUTG = sem("sGP"), sem("sOUT"), sem("sOUTG")

        gy = b32y[:, C_GY : C_GY + H]
        bfe = b32y[:, C_BF : C_BF + 2]
        npc = lambda b, c: b32y[:, C_PC + 2 * b + c : C_PC + 2 * b + c + 1]
        gx = b32x[:, :]
        w1 = b16a[:3, C_W1 : C_W1 + 64]
        x0 = b16a[:3, C_X0 : C_X0 + BS * N]
        w2 = b16b[:65, C_W2 : C_W2 + HID]
        wfold = b16b[:, C_WF : C_WF + 2]

        with nc.Block() as block:

            @block.gpsimd
            def _(gp):
                gp.memset(warm[:], 0.0).then_inc(sGP)  # GP1: dummy-act operand
                gp.memset(h1[64:65, :], 1.0).then_inc(sGP)  # GP2: layer-2 bias row
                gp.memset(sigb[:], sig_half_bias).then_inc(sGP)  # GP3
                # valid mask, fully off the DVE critical path: GP4..7 / GP8..11
                gp.wait_ge(sBY, 16)
                for b in range(BS):
                    npx, npy = npc(b, 0), npc(b, 1)
                    nc.gpsimd.tensor_scalar(
                        vc[b][:, 0:1], npx, -1.0, npx, AL.mult, AL.max
                    ).then_inc(sGP)
                    nc.gpsimd.tensor_scalar(
                        vc[b][:, 1:2], npy, -1.0, npy, AL.mult, AL.max
                    ).then_inc(sGP)
                    gp.wait_ge(sGP, 5 + 4 * b)
                    nc.gpsimd.tensor_scalar(
                        vc[b][:, 0:1], vc[b][:, 0:1], vc[b][:, 1:2], None, AL.max
                    ).then_inc(sGP)
                    gp.wait_ge(sGP, 6 + 4 * b)
                    nc.gpsimd.tensor_scalar(
                        vc[b][:, 0:1], vc[b][:, 0:1], 1e-8, c0, AL.is_gt, AL.mult
                    ).then_inc(sGP)
                # batch-0 field DMAs (SWDGE path, parallel to HWDGE)
                gp.wait_ge(sACT, 10)  # fields tanh b0 done
                gp.dma_start(
                    out=out_d[0, :, 512:1024], in_=obuf[0][:, 512:1024]
                ).then_inc(sOUTG, 16)
                gp.dma_start(
                    out=out_d[0, :, 1024:1536], in_=obuf[0][:, 1024:1536]
                ).then_inc(sOUTG, 16)

            @block.scalar
            def _(act):
                act.dma_start(out=b32y[:], in_=by_d[:]).then_inc(sBY, 16)
                # pre-trigger the exp_and_others ACT_TABLE_LOAD while the
                # input DMAs are in flight
                act.wait_ge(sGP, 1)
                nc.scalar.activation(warm[:], warm[:], ACT.Exp)
                act.wait_ge(sBY, 16)
                # per-batch: sqy(1+4b), sqx(2+4b), ey(3+4b), exa(4+4b) — batch 0's
                # matmul operands (ey half + Ex half) complete ~1.5us earlier
                for b in range(BS):
                    nc.scalar.activation(
                        tys[:, b * H : (b + 1) * H], gy, ACT.Square, bias=npc(b, 1)
                    ).then_inc(sACT)
                    if b == 0:
                        act.wait_ge(sBX, 16)
                    nc.scalar.activation(
                        txs[:, b * W : (b + 1) * W], gx, ACT.Square, bias=npc(b, 0)
                    ).then_inc(sACT)
                    act.wait_ge(sACT, 2 + 4 * b)  # squares landed (same-engine RAW)
                    nc.scalar.activation(
                        ey[:, b * H : (b + 1) * H],
                        tys[:, b * H : (b + 1) * H],
                        ACT.Exp,
                        scale=neg_inv2bw2,
                    ).then_inc(sACT)
                    nc.scalar.activation(
                        exa[b][:, 0:W],
                        txs[:, b * W : (b + 1) * W],
                        ACT.Exp,
                        scale=neg_inv2bw2,
                    ).then_inc(sACT)
                # epilogue: sigmoid(s*x - thr) = 0.5*tanh(s/2*x - thr/2) + 0.5
                act.wait_ge(sGP, 3)
                for b in range(BS):  # densT: 9/11, fieldsT: 10/12
                    act.wait_ge(sPE, 5 + 2 * b)  # dens matmuls of batch b
                    nc.scalar.activation(
                        obuf[b][:, 0:512],
                        psR[b][:, 0:512],
                        ACT.Tanh,
                        scale=sig_half_scale,
                        bias=sigb[:],
                    ).then_inc(sACT)
                    act.wait_ge(sPE, 6 + 2 * b)  # field matmuls of batch b
                    nc.scalar.activation(
                        obuf[b][:, 512:1536], psR[b][:, 512:1536], ACT.Tanh
                    ).then_inc(sACT)
                if True:  # ship f1(b1) from here: no cross-engine hop at the tail
                    act.wait_ge(sACT, 12)
                    act.dma_start(
                        out=out_d[1, :, 1024:1280], in_=obuf[1][:, 1024:1280]
                    ).then_inc(sOUT, 16)
                    act.dma_start(
                        out=out_d[1, :, 1280:1536], in_=obuf[1][:, 1280:1536]
                    ).then_inc(sOUT, 16)

            @block.vector
            def _(dve):
                dve.wait_ge(sPE, 1)
                nc.vector.tensor_scalar(
                    h1[:64, :], psA[:64, :], 0.0, None, AL.max
                ).then_inc(sDVE)  # D1: relu1
                dve.wait_ge(sPE, 2)
                nc.vector.tensor_scalar(h2[:], psB[:], 0.0, None, AL.max).then_inc(
                    sDVE
                )  # D2: relu2
                dve.wait_ge(sBY, 16)
                for b in range(BS):  # D3,D4 / D5,D6
                    dve.wait_ge(sPE, 3 + b)
                    nc.vector.tensor_tensor(
                        vb[b][:], psA[:N, 4 * b : 4 * b + 2], bfe, AL.add
                    ).then_inc(sDVE)
                    dve.wait_ge(sDVE, 3 + 2 * b)
                    dve.wait_ge(sGP, 7 + 4 * b)  # vc[b] ready (on gpsimd)
                    nc.vector.tensor_scalar(
                        ub[b][:], vb[b][:], vc[b][:, 0:1], None, AL.mult
                    ).then_inc(sDVE)
                for b in range(BS):  # D7,D8 / D9,D10
                    dve.wait_ge(sDVE, 4 + 2 * b)  # ub[b] landed
                    dve.wait_ge(sACT, 4 + 4 * b)
                    nc.vector.tensor_scalar(
                        exa[b][:, W : 2 * W],
                        exa[b][:, 0:W],
                        ub[b][:, 0:1],
                        None,
                        AL.mult,
                    ).then_inc(sDVE)
                    nc.vector.tensor_scalar(
                        exu1[b][:], exa[b][:, 0:W], ub[b][:, 1:2], None, AL.mult
                    ).then_inc(sDVE)
                # dens sigmoid fixups
                dve.wait_ge(sACT, 9)  # dens tanh b0 done (densT0)
                nc.vector.tensor_scalar(
                    obuf[0][:, 0:512], obuf[0][:, 0:512], 0.5, 0.5, AL.mult, AL.add
                ).then_inc(sDVE)  # D11: fix0
                dve.wait_ge(sACT, 11)  # dens tanh b1 done (densT1)
                nc.vector.tensor_scalar(
                    obuf[1][:, 0:512], obuf[1][:, 0:512], 0.5, 0.5, AL.mult, AL.add
                ).then_inc(sDVE)  # D12: fix1

            @block.tensor
            def _(pe):
                pe.wait_ge(sBA, 16)
                nc.tensor.matmul(psA[:64, :], w1, x0).then_inc(sPE)  # P1
                pe.wait_ge(sDVE, 1)
                pe.wait_ge(sGP, 2)
                pe.wait_ge(sBB, 16)
                nc.tensor.matmul(psB[:], w2, h1[:]).then_inc(sPE)  # P2
                for b in range(BS):  # P3, P4: vecs (pre-bias) via folded Wf@W3
                    # psA bank0 is read by DVE (relu1, vb[b-1]) — wait for the
                    # read to finish before writing the same bank (P10 hazard)
                    pe.wait_ge(sDVE, 2 + b)
                    nc.tensor.matmul(
                        psA[:N, 4 * b : 4 * b + 2], h2[:, b * N : (b + 1) * N], wfold
                    ).then_inc(sPE)
                for b in range(BS):
                    pe.wait_ge(sACT, 4 + 4 * b)  # ey[b] + Ex[b] (ACT only!)
                    for ch in range(2):  # dens, not gated on the u-products
                        lhs = ey[:, b * H + ch * 128 : b * H + (ch + 1) * 128]
                        mm = nc.tensor.matmul(
                            psR[b][:, ch * W : (ch + 1) * W], lhs, exa[b][:, 0:W]
                        )
                        if ch == 1:
                            mm.then_inc(sPE)  # -> 5 + 2b
                    pe.wait_ge(sDVE, 8 + 2 * b)  # exa[b] U-half + exu1[b]
                    for ch in range(2):  # f0
                        lhs = ey[:, b * H + ch * 128 : b * H + (ch + 1) * 128]
                        nc.tensor.matmul(
                            psR[b][:, 512 + ch * W : 512 + (ch + 1) * W],
                            lhs,
                            exa[b][:, W : 2 * W],
                        )
                    for ch in range(2):  # f1
                        lhs = ey[:, b * H + ch * 128 : b * H + (ch + 1) * 128]
                        mm = nc.tensor.matmul(
                            psR[b][:, 1024 + ch * W : 1024 + (ch + 1) * W],
                            lhs,
                            exu1[b][:],
                        )
                        if ch == 1:
                            mm.then_inc(sPE)  # -> 6 + 2b (all four field matmuls)

            @block.sync
            def _(sp):
                sp.dma_start(out=b16a[:], in_=ba_d[:]).then_inc(sBA, 16)
                sp.dma_start(out=b32x[:], in_=bx_d[:]).then_inc(sBX, 16)
                sp.dma_start(out=b16b[:], in_=bb_d[:]).then_inc(sBB, 16)
                # dens b0 (SP is idle; avoids the gpsimd hop), then batch 1
                sp.wait_ge(sDVE, 11)  # dens fixup b0 done
                sp.dma_start(out=out_d[0, :, 0:512], in_=obuf[0][:, 0:512]).then_inc(
                    sOUT, 16
                )
                sp.wait_ge(sDVE, 12)  # dens fixup b1 done
                sp.dma_start(out=out_d[1, :, 0:512], in_=obuf[1][:, 0:512]).then_inc(
                    sOUT, 16
                )
                sp.wait_ge(sACT, 12)  # fields tanh b1 done
                sp.dma_start(
                    out=out_d[1, :, 512:1024], in_=obuf[1][:, 512:1024]
                ).then_inc(sOUT, 16)
                sp.wait_ge(sOUT, 16 * 5)
                sp.wait_ge(sOUTG, 16 * 2)

    return nc


def kernel(
    bezier_points,
    W1,
    b1,
    W2,
    b2,
    W3,
    b3,
    Wf,
    bf,
    kde_bandwidth,
    density_threshold,
    trace=False,
):
    global LAST_RESULT
    f32, f16 = np.float32, np.float16
    pts = np.asarray(bezier_points, f32).reshape(B, N, 2)
    W1, b1 = np.asarray(W1, f32), np.asarray(b1, f32)
    W2, b2 = np.asarray(W2, f32), np.asarray(b2, f32)
    W3, b3 = np.asarray(W3, f32), np.asarray(b3, f32)
    Wf, bf = np.asarray(Wf, f32), np.asarray(bf, f32)

    bw = max(float(np.float32(kde_bandwidth)), 1e-5)
    thr = float(np.float32(density_threshold))
    neg_inv2bw2 = -1.0 / (2.0 * bw * bw)
    c0 = math.exp(-1e-8 / (2.0 * bw * bw))
    s = 1.0 / (N * bw * math.sqrt(2.0 * math.pi))
    nc = _build(neg_inv2bw2, c0, 0.5 * s, -0.5 * thr)

    # host-side input marshalling: grid constants, weight transposes, and
    # folding layer 3 into the field projection
    wfold = (Wf @ W3).astype(f32)  # [2, HID]
    bf_eff = (bf + Wf @ b3).astype(f32)  # [2]

    bloby = np.zeros((128, NCY), f32)
    bloby[:, C_GY : C_GY + H] = np.linspace(-1.0, 1.0, H, dtype=f32)
    bloby[:, C_BF : C_BF + 2] = bf_eff
    blobx = np.ascontiguousarray(
        np.broadcast_to(np.linspace(-1.0, 1.0, W, dtype=f32), (128, W))
    )
    bloba = np.zeros((3, NCA), f16)
    bloba[:, C_W1 : C_W1 + 64] = np.vstack([W1.T, b1[None, :]])
    blobb = np.zeros((128, NCB), f16)
    blobb[:65, C_W2 : C_W2 + HID] = np.vstack([W2.T, b2[None, :]])
    blobb[:, C_WF : C_WF + 2] = wfold.T

    in_maps = []
    for i in range(NCORES):
        sh = pts[i * BS : (i + 1) * BS]  # [BS, N, 2]
        cy = bloby.copy()
        for b in range(BS):
            cy[:, C_PC + 2 * b] = -sh[b, :, 0]
            cy[:, C_PC + 2 * b + 1] = -sh[b, :, 1]
        ca = bloba.copy()
        ca[0, C_X0 : C_X0 + BS * N] = sh[..., 0].reshape(-1)
        ca[1, C_X0 : C_X0 + BS * N] = sh[..., 1].reshape(-1)
        ca[2, C_X0 : C_X0 + BS * N] = 1.0
        in_maps.append({"b32y": cy, "b32x": blobx, "b16a": ca, "b16b": blobb})

    res = run_bass_kernel_spmd(nc, in_maps, list(range(NCORES)), trace=trace)
    LAST_RESULT = res

    density = np.empty((B, 1, H, W), f32)
    field = np.empty((B, 2, H, W), f32)
    for i in range(NCORES):
        scr = res.results[i]["out_o"]  # [BS, 128, 1536]
        maps = scr.reshape(BS, 128, 3, 2, W).transpose(2, 0, 3, 1, 4)
        # maps[m, b, ch, p, w] with h = ch*128 + p
        density[i * BS : (i + 1) * BS, 0] = maps[0].reshape(BS, H, W)
        field[i * BS : (i + 1) * BS, 0] = maps[1].reshape(BS, H, W)
        field[i * BS : (i + 1) * BS, 1] = maps[2].reshape(BS, H, W)
    return density, field
